# revision 11
# baseline (speedup 1.0000x reference)
"""MoE layer (nn_MoELayer_28260884807815) on 8 Trainium2 NeuronCores.

Strategy: data-parallel over tokens (B*S = 8192 -> 1024 tokens/core),
weights replicated, no cross-core communication. All compute on device:
gating MLP -> top-2 -> renormalized combine weights -> expert matmuls
(combine weights applied as per-partition scalars on the expert outputs)
-> relu -> output projection.

Matmuls run in float32r (full-rate fp32 on TRN2 when moving dim >= 256).
"""

import sys

sys.path.insert(0, "/opt/trn_rl_repo")

import numpy as np

import concourse.bass as bass
import concourse.mybir as mybir
import concourse.tile as tile
from concourse import bacc
from concourse.bass_utils import run_bass_kernel_spmd
from concourse.masks import make_identity

F32 = mybir.dt.float32
F32R = mybir.dt.float32r
AF = mybir.ActivationFunctionType
OP = mybir.AluOpType

B, S, D, H, E = 4, 2048, 1024, 2048, 8
GH, GH2, GQ = 512, 256, 128
NCORES = 8
T = (B * S) // NCORES  # 1024 tokens per core
TT = T // 128  # 8 token tiles
DK = D // 128  # 8 contraction tiles over D
HC = 4  # h chunks of 512
HS = H // HC  # 512


def build_nc():
    nc = bacc.Bacc("TRN2", target_bir_lowering=False, debug=False,
                   num_devices=NCORES)

    x = nc.dram_tensor("x", [T, D], F32, kind="ExternalInput")
    ftoh = nc.dram_tensor("ftoh", [3, T], F32, kind="ExternalInput")
    gw1 = nc.dram_tensor("gw1", [D, GH], F32R, kind="ExternalInput")
    gb1 = nc.dram_tensor("gb1", [GH], F32, kind="ExternalInput")
    gw2 = nc.dram_tensor("gw2", [GH, GH2], F32R, kind="ExternalInput")
    gb2 = nc.dram_tensor("gb2", [GH2], F32, kind="ExternalInput")
    gw3 = nc.dram_tensor("gw3", [GH2, E], F32, kind="ExternalInput")
    gb3 = nc.dram_tensor("gb3", [E], F32, kind="ExternalInput")
    temb = nc.dram_tensor("temb", [3, GQ], F32, kind="ExternalInput")
    tw = nc.dram_tensor("tw", [GQ, E], F32, kind="ExternalInput")
    tb = nc.dram_tensor("tb", [E], F32, kind="ExternalInput")
    ew = nc.dram_tensor("ew", [E, D, H], F32R, kind="ExternalInput")
    eb = nc.dram_tensor("eb", [E, H], F32R, kind="ExternalInput")
    ow = nc.dram_tensor("ow", [H, D], F32R, kind="ExternalInput")
    ob = nc.dram_tensor("ob", [D], F32, kind="ExternalInput")
    out = nc.dram_tensor("out", [T, D], F32, kind="ExternalOutput")

    with tile.TileContext(nc) as tc:
        with tc.tile_pool(name="const", bufs=1) as cpool:
            ident = cpool.tile([128, 128], F32)
            make_identity(nc, ident)
            ones1 = cpool.tile([1, 128], F32)
            nc.vector.memset(ones1, 1.0)

            gb1_s = cpool.tile([128, GH // 128], F32)
            nc.sync.dma_start(gb1_s, gb1.rearrange("(m p) -> p m", p=128))
            gb2_s = cpool.tile([128, GH2 // 128], F32)
            nc.sync.dma_start(gb2_s, gb2.rearrange("(m p) -> p m", p=128))
            tbgb3 = cpool.tile([1, E], F32)
            gb3_s = cpool.tile([1, E], F32)
            nc.sync.dma_start(tbgb3, tb[None, :])
            nc.sync.dma_start(gb3_s, gb3[None, :])
            nc.vector.tensor_add(tbgb3, tbgb3, gb3_s)
            tw_s = cpool.tile([GQ, E], F32)
            nc.sync.dma_start(tw_s, tw[:])
            temb_s = cpool.tile([3, GQ], F32)
            nc.sync.dma_start(temb_s, temb[:])
            eb_s = cpool.tile([E, H], F32R)
            nc.sync.dma_start(eb_s, eb[:])
            ob_s = cpool.tile([1, D], F32)
            nc.sync.dma_start(ob_s, ob[None, :])
            # one-hot of feature type, class on partitions: oh[c, t]
            oh = cpool.tile([3, T], F32)
            nc.sync.dma_start(oh, ftoh[:])

            # etb[c, e] = type_emb[c] @ tw + (tb + gb3)
            etb = cpool.tile([3, E], F32)
            with tc.tile_pool(name="etb_ps", bufs=1, space="PSUM") as pp:
                teT_ps = pp.tile([GQ, 3], F32)
                nc.tensor.transpose(teT_ps, temb_s, ident[:3, :3])
                teT = cpool.tile([GQ, 3], F32)
                nc.scalar.copy(teT, teT_ps)
                etb_ps = pp.tile([3, E], F32)
                nc.tensor.matmul(etb_ps, teT, tw_s, start=True, stop=False)
                nc.tensor.matmul(etb_ps, ones1[:, :3], tbgb3,
                                 start=False, stop=True)
                nc.scalar.copy(etb, etb_ps)

            with tc.tile_pool(name="big", bufs=1) as big:
                xT = big.tile([128, DK, T], F32R)  # x transposed: [d, k, t]
                acc = big.tile([128, TT, H], F32)  # combined expert out
                cw = big.tile([128, TT, E], F32)  # combine weights
                cwT = big.tile([E, T], F32R)

                # ---- transpose x into xT ----
                with tc.tile_pool(name="xn", bufs=3) as xn, \
                     tc.tile_pool(name="xps", bufs=4, space="PSUM") as xps:
                    for t in range(TT):
                        xnat = xn.tile([128, D], F32)
                        nc.sync.dma_start(xnat, x[t * 128:(t + 1) * 128, :])
                        for k in range(DK):
                            ps = xps.tile([128, 128], F32)
                            nc.tensor.transpose(
                                ps, xnat[:, k * 128:(k + 1) * 128], ident)
                            nc.scalar.copy(xT[:, k, t * 128:(t + 1) * 128], ps)

                # ---- gating MLP (feature-major activations) ----
                with tc.tile_pool(name="gate", bufs=1) as gp, \
                     tc.tile_pool(name="gw", bufs=2) as gwp, \
                     tc.tile_pool(name="gps", bufs=2, space="PSUM") as gps:
                    h1T = gp.tile([128, GH // 128, T], F32R)
                    h2T = gp.tile([128, GH2 // 128, T], F32)
                    for m in range(GH // 128):
                        w1s = gwp.tile([128, DK, 128], F32R, tag="w1s")
                        nc.sync.dma_start(
                            w1s, gw1[:, m * 128:(m + 1) * 128].rearrange(
                                "(k p) f -> p k f", p=128))
                        for n in range(T // 512):
                            ps = gps.tile([128, 512], F32, tag="g1")
                            for k in range(DK):
                                nc.tensor.matmul(
                                    ps, w1s[:, k, :],
                                    xT[:, k, n * 512:(n + 1) * 512],
                                    start=(k == 0), stop=(k == DK - 1))
                            nc.scalar.activation(
                                h1T[:, m, n * 512:(n + 1) * 512], ps,
                                AF.Relu, bias=gb1_s[:, m:m + 1])
                    for m in range(GH2 // 128):
                        w2s = gwp.tile([128, GH // 128, 128], F32R, tag="w2s")
                        nc.sync.dma_start(
                            w2s, gw2[:, m * 128:(m + 1) * 128].rearrange(
                                "(k p) f -> p k f", p=128))
                        for n in range(T // 512):
                            ps = gps.tile([128, 512], F32, tag="g2")
                            for k in range(GH // 128):
                                nc.tensor.matmul(
                                    ps, w2s[:, k, :],
                                    h1T[:, k, n * 512:(n + 1) * 512],
                                    start=(k == 0), stop=(k == GH // 128 - 1))
                            nc.scalar.activation(
                                h2T[:, m, n * 512:(n + 1) * 512], ps,
                                AF.Relu, bias=gb2_s[:, m:m + 1])

                    w3s = gp.tile([128, GH2 // 128, E], F32)
                    nc.sync.dma_start(
                        w3s, gw3.rearrange("(k p) f -> p k f", p=128))

                    # logits + type bias per token tile, tokens on partitions
                    for t in range(TT):
                        ps = gps.tile([128, E], F32, tag="lg")
                        for k in range(GH2 // 128):
                            nc.tensor.matmul(
                                ps, h2T[:, k, t * 128:(t + 1) * 128],
                                w3s[:, k, :], start=(k == 0), stop=False)
                        nc.tensor.matmul(
                            ps, oh[:, t * 128:(t + 1) * 128], etb,
                            start=False, stop=True)
                        g = gp.tile([128, E], F32, tag="g")
                        nc.scalar.copy(g, ps)

                        mx = gp.tile([128, 8], F32, tag="mx")
                        nc.vector.max(mx, g)
                        # w1 = 1/(1+exp(l2-l1)), w2 = 1-w1 (exact renorm
                        # of top-2 softmax probs)
                        d21 = gp.tile([128, 1], F32, tag="d21")
                        nc.vector.tensor_sub(d21, mx[:, 1:2], mx[:, 0:1])
                        e2 = gp.tile([128, 1], F32, tag="e2")
                        nc.scalar.activation(e2, d21, AF.Exp)
                        den = gp.tile([128, 1], F32, tag="den")
                        nc.vector.tensor_scalar_add(den, e2, 1.0)
                        w1 = gp.tile([128, 1], F32, tag="w1")
                        nc.vector.reciprocal(w1, den)
                        w2 = gp.tile([128, 1], F32, tag="w2")
                        nc.vector.tensor_mul(w2, e2, w1)
                        w1m2 = gp.tile([128, 1], F32, tag="w1m2")
                        nc.vector.tensor_sub(w1m2, w1, w2)
                        m1 = gp.tile([128, E], F32, tag="m1")
                        nc.vector.tensor_scalar(m1, g, mx[:, 0:1], None,
                                                op0=OP.is_ge)
                        m12 = gp.tile([128, E], F32, tag="m12")
                        nc.vector.tensor_scalar(m12, g, mx[:, 1:2], None,
                                                op0=OP.is_ge)
                        # cw = m1*(w1-w2) + m12*w2
                        t12 = gp.tile([128, E], F32, tag="t12")
                        nc.vector.tensor_scalar_mul(t12, m12, w2)
                        nc.vector.scalar_tensor_tensor(
                            cw[:, t, :], m1, w1m2, t12,
                            op0=OP.mult, op1=OP.add)
                        psT = gps.tile([8, 128], F32, tag="cwT")
                        nc.tensor.transpose(psT, cw[:, t, :], ident)
                        nc.scalar.copy(cwT[:, t * 128:(t + 1) * 128], psT)

                # ---- dense expert compute with combine ----
                with tc.tile_pool(name="ewp", bufs=3) as ewp, \
                     tc.tile_pool(name="eps", bufs=3, space="PSUM") as eps, \
                     tc.tile_pool(name="bps", bufs=2, space="PSUM") as bps:
                    for hc in range(HC):
                        hs = slice(hc * HS, (hc + 1) * HS)
                        # init acc with sum_e cw[t,e]*eb[e,h]
                        for t in range(TT):
                            psb = bps.tile([128, HS], F32, tag="psb")
                            nc.tensor.matmul(
                                psb, cwT[:, t * 128:(t + 1) * 128],
                                eb_s[:, hs], start=True, stop=True)
                            nc.scalar.copy(acc[:, t, hs], psb)
                        for e in range(E):
                            ewb = ewp.tile([128, DK, HS], F32R, tag="ewb")
                            nc.sync.dma_start(
                                ewb, ew[e, :, hs].rearrange(
                                    "(k p) h -> p k h", p=128))
                            for t in range(TT):
                                ps = eps.tile([128, HS], F32, tag="py")
                                for k in range(DK):
                                    nc.tensor.matmul(
                                        ps, xT[:, k, t * 128:(t + 1) * 128],
                                        ewb[:, k, :],
                                        start=(k == 0), stop=(k == DK - 1))
                                # acc += cw[:,t,e] * ps
                                nc.vector.scalar_tensor_tensor(
                                    acc[:, t, hs], ps, cw[:, t, e:e + 1],
                                    acc[:, t, hs], op0=OP.mult, op1=OP.add)

                # ---- final projection: out = relu(acc) @ ow + ob ----
                with tc.tile_pool(name="owp", bufs=1) as owp, \
                     tc.tile_pool(name="atp", bufs=2) as atp, \
                     tc.tile_pool(name="outp", bufs=3) as outp, \
                     tc.tile_pool(name="ops", bufs=2, space="PSUM") as ops, \
                     tc.tile_pool(name="tps", bufs=4, space="PSUM") as tps:
                    HK = H // 128  # 16
                    owb = owp.tile([128, HK, D], F32R)
                    nc.sync.dma_start(
                        owb, ow.rearrange("(k p) f -> p k f", p=128))
                    for t in range(TT):
                        aT = atp.tile([128, HK, 128], F32R, tag="aT")
                        for k in range(HK):
                            tp = tps.tile([128, 128], F32, tag="tp")
                            nc.tensor.transpose(
                                tp, acc[:, t, k * 128:(k + 1) * 128],
                                ident)
                            nc.scalar.activation(aT[:, k, :], tp, AF.Relu)
                        for dc in range(D // 512):
                            ds_ = slice(dc * 512, (dc + 1) * 512)
                            po = ops.tile([128, 512], F32, tag="po")
                            for k in range(HK):
                                nc.tensor.matmul(
                                    po, aT[:, k, :], owb[:, k, ds_],
                                    start=(k == 0), stop=False)
                            nc.tensor.matmul(po, ones1, ob_s[:, ds_],
                                             start=False, stop=True)
                            ot = outp.tile([128, 512], F32, tag="ot")
                            nc.scalar.copy(ot, po)
                            nc.sync.dma_start(
                                out[t * 128:(t + 1) * 128, ds_], ot)

    nc.compile()
    return nc


_NC_CACHE = {}


def _get_nc():
    if "nc" not in _NC_CACHE:
        _NC_CACHE["nc"] = build_nc()
    return _NC_CACHE["nc"]


def kernel(x, feature_types, gw1, gb1, gw2, gb2, gw3, gb3, type_emb, tw, tb,
           ew, eb, ow, ob):
    nc = _get_nc()
    x = np.ascontiguousarray(np.asarray(x, dtype=np.float32)).reshape(B * S, D)
    fti = np.asarray(feature_types).reshape(B * S).astype(np.int64)
    ftoh = (fti[None, :] == np.arange(3)[:, None]).astype(np.float32)

    shared = {
        "gw1": np.asarray(gw1, np.float32),
        "gb1": np.asarray(gb1, np.float32),
        "gw2": np.asarray(gw2, np.float32),
        "gb2": np.asarray(gb2, np.float32),
        "gw3": np.asarray(gw3, np.float32),
        "gb3": np.asarray(gb3, np.float32),
        "temb": np.asarray(type_emb, np.float32),
        "tw": np.asarray(tw, np.float32),
        "tb": np.asarray(tb, np.float32),
        "ew": np.ascontiguousarray(np.asarray(ew, np.float32)),
        "eb": np.asarray(eb, np.float32),
        "ow": np.ascontiguousarray(np.asarray(ow, np.float32)),
        "ob": np.asarray(ob, np.float32),
    }
    in_maps = []
    for c in range(NCORES):
        m = dict(shared)
        m["x"] = x[c * T:(c + 1) * T]
        m["ftoh"] = np.ascontiguousarray(ftoh[:, c * T:(c + 1) * T])
        in_maps.append(m)

    res = run_bass_kernel_spmd(nc, in_maps, list(range(NCORES)))
    out = np.concatenate([res.results[c]["out"] for c in range(NCORES)],
                         axis=0)
    return out.reshape(B, S, D)
